# revision 66
# baseline (speedup 1.0000x reference)
"""Linear-attention block (elu+1 feature map) for Trainium2, 8-core SPMD.

Data-parallel over batch: each of the 8 cores processes 4 of the 32 batches
with fully replicated weights; no collectives. All heavy matmuls run in bf16
with fp32 PSUM accumulation.

Layout strategy per core (B=4 local batches, N=512 tokens, D=1024, H=16,
Dk=64; heads processed as 8 "pairs" of 2 heads = 128 d_model dims):
  - x [128t,1024d] tiles -> cast bf16 -> DMA-xbar-transpose -> xT [d,t]
  - qT = W_q.T @ x.T   (W-chunk stationary)          [o,t] transposed
  - k,v = x @ W        (xT-chunk stationary)         [t,o] natural
  - biases via K=1 ones-row matmuls; b_v exactly folded into
    b_o2 = b_v @ W_o + b_o (the eps-correction term is ~1e-11, below fp32)
  - phi(y) = elu(y)+1 = min(exp(y),1) + relu(y): ACT exp + 2 DVE ops
  - per pair: kv[d,e]+ksum[d] one PSUM tile; numerator via two K=64
    tile_position matmuls on kv diagonal blocks; denominator via
    diag-masked ksum2 [128,2] matmul; 1/denom broadcast rows->128
    partitions with a constant 2x128 mask matmul; one DVE multiply
  - final = attn_outT.T @ W_o + b_o2 -> natural [t,o] -> DMA out
"""

import os
from contextlib import ExitStack

DEBUG = bool(int(os.environ.get("KERNEL_DEBUG", "0")))

import numpy as np

import concourse.bacc as bacc
import concourse.bass as bass
import concourse.mybir as mybir
import concourse.tile as tile
from concourse._compat import with_exitstack
from concourse.bass_utils import run_bass_kernel_spmd

FP32 = mybir.dt.float32
BF16 = mybir.dt.bfloat16
AF = mybir.ActivationFunctionType
OP = mybir.AluOpType

N_CORES = 8
B_LOC = 4          # batches per core
N = 512            # sequence length
D = 1024           # d_model
NT = 4             # token tiles of 128 per batch
NK = 8             # d_model chunks of 128
EPS = 1e-6


@with_exitstack
def _emit(ctx: ExitStack, tc: "tile.TileContext", io: dict):
    nc = tc.nc
    x_d = io["x"]
    out_d = io["out"]

    # ---------------- pools ----------------
    const = ctx.enter_context(tc.tile_pool(name="const", bufs=1))
    xbf_p = ctx.enter_context(tc.tile_pool(name="xbf", bufs=2))
    xT_p = ctx.enter_context(tc.tile_pool(name="xT", bufs=2))
    qfT_p = ctx.enter_context(tc.tile_pool(name="qfT", bufs=2))
    kf_p = ctx.enter_context(tc.tile_pool(name="kf", bufs=2))
    v_p = ctx.enter_context(tc.tile_pool(name="v", bufs=2))
    aT_p = ctx.enter_context(tc.tile_pool(name="aT", bufs=2))
    er_p = ctx.enter_context(tc.tile_pool(name="er", bufs=3))
    kvs_p = ctx.enter_context(tc.tile_pool(name="kvs", bufs=6))
    ks2_p = ctx.enter_context(tc.tile_pool(name="ks2", bufs=6))
    rc_p = ctx.enter_context(tc.tile_pool(name="rc", bufs=2))
    rcb_p = ctx.enter_context(tc.tile_pool(name="rcb", bufs=5))
    nsb_p = ctx.enter_context(tc.tile_pool(name="nsb", bufs=5))
    xstage_p = ctx.enter_context(tc.tile_pool(name="xstage", bufs=5))
    osb_p = ctx.enter_context(tc.tile_pool(name="osb", bufs=2))
    stage_p = ctx.enter_context(tc.tile_pool(name="wstage", bufs=2))
    ps = ctx.enter_context(tc.tile_pool(name="ps", bufs=6, space="PSUM"))
    pskv = ctx.enter_context(tc.tile_pool(name="pskv", bufs=2, space="PSUM"))

    # ---------------- constants ----------------
    ones_row = const.tile([1, 512], BF16, tag="ones_row")
    nc.vector.memset(ones_row[:], 1.0)
    one1 = const.tile([1, 1], BF16, tag="one1")
    nc.vector.memset(one1[:], 1.0)
    import ml_dtypes

    bf = ml_dtypes.bfloat16
    mask_np = np.zeros((2, 128), bf)
    mask_np[0, 0:64] = 1.0
    mask_np[1, 64:128] = 1.0
    mask2 = const.tile([2, 128], BF16, tag="mask2")
    nc.sync.dma_start(mask2[:], nc.inline_tensor(mask_np, name="mask2c").ap())
    # block-diagonal head mask [128, 128] and its [128, 2] column form
    mdiag_np = np.zeros((128, 128), bf)
    mdiag_np[0:64, 0:64] = 1.0
    mdiag_np[64:128, 64:128] = 1.0
    mdiag = const.tile([128, 128], BF16, tag="mdiag")
    nc.sync.dma_start(mdiag[:], nc.inline_tensor(mdiag_np, name="mdiagc").ap())
    m2t_np = np.zeros((128, 2), bf)
    m2t_np[0:64, 0] = 1.0
    m2t_np[64:128, 1] = 1.0
    mask2T = const.tile([128, 2], BF16, tag="mask2T")
    nc.sync.dma_start(mask2T[:], nc.inline_tensor(m2t_np, name="mask2Tc").ap())

    # ---------------- input loads ----------------
    # x: fp32 over HWDGE in halves + DVE cast (keeps the SWDGE path free for
    # W_q/W_k, which gate batch-0 compute)
    def load_x(b):
        xT = xT_p.tile([128, NK, 512], BF16, tag="xT", name=f"xT{b}")
        for t in range(NT):
            xb = xbf_p.tile([128, D], BF16, tag="xbf", name=f"xb{b}_{t}")
            for h in range(2):
                xs = xstage_p.tile([128, 512], FP32, tag="xstage",
                                   name=f"xs{b}_{t}_{h}")
                nc.sync.dma_start(
                    xs[:],
                    x_d[b, t * 128:(t + 1) * 128, h * 512:(h + 1) * 512],
                )
                nc.vector.tensor_copy(xb[:, h * 512:(h + 1) * 512], xs[:])
            # xbar transpose with partition-fold: out[p, c, t] = xb[t, c*128+p]
            # (on the ACT hwdge queue — the sync queue's serial descriptor
            # generation for big DMAs would delay these)
            nc.scalar.dma_start(
                xT[:, :, t * 128:(t + 1) * 128], xb[:], transpose=True
            )
        return xT

    def load_row(key):
        r = const.tile([1, D], BF16, tag=f"row_{key}", name=f"row_{key}")
        nc.gpsimd.dma_start(r[:], io[key][:])
        return r

    bk_row = load_row("bk")
    bo_row = load_row("bo")
    bv_cols = const.tile([128, NK], BF16, tag="bvc")
    nc.gpsimd.dma_start(bv_cols[:], io["bv"].rearrange("c p -> p c"))
    # b_q as column chunks (fp32) for ACT per-partition bias in the q phi
    bq_cols = const.tile([128, NK], FP32, tag="bqc")
    nc.gpsimd.dma_start(bq_cols[:], io["bq"].rearrange("r (c p) -> p (r c)", p=128))

    # Weights: W_q/W_k/W_o stream via gpsimd cast-DMAs; W_v goes fp32 over
    # HWDGE with DVE casts (parallel paths). Only W_q + x gate batch-0, so
    # the other loads are emitted later (= lower scheduler priority) to
    # leave them the HBM bandwidth.
    W_bf = {
        name: const.tile([128, NK, D], BF16, tag=f"W_{name}", name=f"W_{name}")
        for name in ("wq", "wk", "wv", "wo")
    }

    def emit_w_load(name):
        wd = io[name]
        wt = W_bf[name]
        for ki in range(NK):
            # wq is startup-critical: stripe it over both paths
            hwdge = (name == "wv") or (name == "wq" and ki % 2 == 1)
            if hwdge:
                ws = stage_p.tile([128, D], FP32, tag="wstage",
                                  name=f"ws_{name}{ki}")
                nc.sync.dma_start(ws[:], wd[ki * 128:(ki + 1) * 128, :])
                nc.vector.tensor_copy(wt[:, ki, :], ws[:])
            else:
                nc.gpsimd.dma_start(wt[:, ki, :], wd[ki * 128:(ki + 1) * 128, :])

    emit_w_load("wq")
    xT_next = load_x(0)

    bo2_row = const.tile([1, D], BF16, tag="bo2")

    def emit_bo2():
        # b_o2 = b_v @ W_o + b_o (emitted late so it doesn't head-of-line
        # block the in-order PE queue on the W_o DMA)
        for h in range(2):
            pb = ps.tile([128, 512], FP32, tag="ps", name=f"pbo2_{h}")
            for ki in range(NK):
                nc.tensor.matmul(
                    pb[0:1, :],
                    lhsT=bv_cols[:, ki:ki + 1],
                    rhs=W_bf["wo"][:, ki, h * 512:(h + 1) * 512],
                    start=(ki == 0),
                    stop=False,
                )
            nc.tensor.matmul(
                pb[0:1, :], lhsT=one1[:], rhs=bo_row[:, h * 512:(h + 1) * 512],
                start=False, stop=True,
            )
            nc.any.tensor_copy(bo2_row[:, h * 512:(h + 1) * 512], pb[0:1, :])

    # ---------------- phase emitters ----------------
    def q_oc(xT, qfT, oc):
        # one o-chunk of the transposed q projection + phi
        pq = ps.tile([128, 512], FP32, tag="ps", name=f"pq{oc}")
        for ki in range(NK):
            nc.tensor.matmul(
                pq[:],
                lhsT=W_bf["wq"][:, ki, oc * 128:(oc + 1) * 128],
                rhs=xT[:, ki, :],
                start=(ki == 0),
                stop=(ki == NK - 1),
            )
        e = er_p.tile([128, 512], FP32, tag="er", name=f"e{oc}")
        nc.scalar.activation(e[:], pq[:], AF.Exp, bias=bq_cols[:, oc:oc + 1])
        r = er_p.tile([128, 512], FP32, tag="er", name=f"r{oc}")
        nc.scalar.activation(r[:], pq[:], AF.Relu, bias=bq_cols[:, oc:oc + 1])
        nc.vector.scalar_tensor_tensor(
            qfT[:, oc, :], e[:], 1.0, r[:], OP.min, OP.add
        )

    # attention pipeline state (flows across batch boundaries)
    st1 = []  # (p, kvs, ks2, qfT, aT) awaiting num/den matmuls
    st2 = []  # (p, nsb, rcb, aT) awaiting bcast+div

    def numden_pair(pend):
        p, kvs, ks2, qfT, aT = pend
        pnum = ps.tile([128, 512], FP32, tag="ps", name=f"pnum{p}")
        nc.tensor.matmul(
            pnum[:], lhsT=kvs[:], rhs=qfT[:, p, :], start=True, stop=True
        )
        pden = ps.tile([2, 512], FP32, tag="ps", name=f"pden{p}")
        nc.tensor.matmul(
            pden[:], lhsT=ks2[:], rhs=qfT[:, p, :], start=True, stop=True
        )
        # eps (1e-6) dropped: denom = qf.ksum is >= O(100) here, so the
        # eps term is ~1e-9 relative — far below bf16 noise
        rc = rc_p.tile([2, 512], FP32, tag="rc", name=f"rc{p}")
        nc.vector.reciprocal_approx_fast(rc[:], pden[:])
        rcb = rcb_p.tile([2, 512], BF16, tag="rcb", name=f"rcb{p}")
        nc.vector.tensor_copy(rcb[:], rc[:])
        nsb = nsb_p.tile([128, 512], BF16, tag="nsb", name=f"nsb{p}")
        nc.scalar.copy(nsb[:], pnum[:])
        st2.append((p, nsb, rcb, aT))

    def finish_pair(pend):
        p, nsb, rcb, aT = pend
        pbc = ps.tile([128, 512], FP32, tag="ps", name=f"pbc{p}")
        nc.tensor.matmul(
            pbc[:], lhsT=mask2[:], rhs=rcb[:], start=True, stop=True
        )
        nc.vector.tensor_tensor(aT[:, p, :], nsb[:], pbc[:], OP.mult)

    def drain_step():
        if st1:
            numden_pair(st1.pop(0))
        if st2 and (len(st2) > 3 or not st1):
            finish_pair(st2.pop(0))

    def drain_all():
        while st1 or st2:
            drain_step()

    # ---------------- per-batch pipeline ----------------
    qfT = None
    for b in range(B_LOC):
        xT = xT_next
        if b + 1 < B_LOC:
            xT_next = load_x(b + 1)

        if b == 0:
            # ---- q (transposed) + phi (b_q folded into ACT bias) ----
            qfT = qfT_p.tile([128, NK, 512], BF16, tag="qfT", name="qfT0")
            for oc in range(NK):
                q_oc(xT, qfT, oc)
            # remaining weights, emitted after batch-0 q so their DMAs don't
            # steal HBM bandwidth from the startup-critical W_q + x loads
            emit_w_load("wk")
            emit_w_load("wv")
            emit_w_load("wo")

        # ---- k, v (natural) + phi / cast ----
        kf = kf_p.tile([128, NT, D], BF16, tag="kf")
        # v stored pair-strided: per pair p, cols [129p : 129p+128] = v,
        # col 129p+128 = 1.0 (so kv and ksum come from one matmul)
        vv = v_p.tile([128, NT, 8 * 129], BF16, tag="v")
        for t in range(NT):
            ones_cols = vv[:, t, :].rearrange("p (g c) -> p g c", c=129)[:, :, 128]
            nc.vector.memset(ones_cols, 1.0)
            pk = [ps.tile([128, 512], FP32, tag="ps", name=f"pk{h}") for h in range(2)]
            pv = [ps.tile([128, 512], FP32, tag="ps", name=f"pv{h}") for h in range(2)]
            for ki in range(NK):
                lhs = xT[:, ki, t * 128:(t + 1) * 128]
                for h in range(2):
                    nc.tensor.matmul(
                        pk[h][:], lhsT=lhs,
                        rhs=W_bf["wk"][:, ki, h * 512:(h + 1) * 512],
                        start=(ki == 0), stop=False,
                    )
                    nc.tensor.matmul(
                        pv[h][:], lhsT=lhs,
                        rhs=W_bf["wv"][:, ki, h * 512:(h + 1) * 512],
                        start=(ki == 0), stop=(ki == NK - 1),
                    )
            for h in range(2):
                nc.tensor.matmul(
                    pk[h][:], lhsT=ones_row[:, 0:128],
                    rhs=bk_row[:, h * 512:(h + 1) * 512],
                    start=False, stop=True,
                )
                e = er_p.tile([128, 512], FP32, tag="er")
                nc.scalar.activation(e[:], pk[h][:], AF.Exp)
                r = er_p.tile([128, 512], FP32, tag="er")
                nc.scalar.activation(r[:], pk[h][:], AF.Relu)
                nc.vector.scalar_tensor_tensor(
                    kf[:, t, h * 512:(h + 1) * 512], e[:], 1.0, r[:],
                    OP.min, OP.add,
                )
                for pl in range(4):
                    p = h * 4 + pl
                    nc.any.tensor_copy(
                        vv[:, t, p * 129:p * 129 + 128],
                        pv[h][:, pl * 128:(pl + 1) * 128],
                    )

        # ---- attention (8 head-pairs, two-level software pipeline: num/den
        # matmuls lag the kv matmuls by 3 pairs; bcast+div lag 2 more; the
        # tail drains interleaved with the NEXT batch's q projection so the
        # in-order PE queue never waits on ACT/DVE chains) ----
        aT = aT_p.tile([128, NK, 512], BF16, tag="aT")
        for p in range(NK):
            pkv = pskv.tile([128, 129], FP32, tag="pskv", name=f"pkv{p}")
            for t in range(NT):
                lhs = kf[:, t, p * 128:(p + 1) * 128]
                nc.tensor.matmul(
                    pkv[:], lhsT=lhs,
                    rhs=vv[:, t, p * 129:p * 129 + 129],
                    start=(t == 0), stop=(t == NT - 1),
                )
            # kvs = kv * blockdiag mask (kills cross-head cols), bf16
            kvs = kvs_p.tile([128, 128], BF16, tag="kvs", name=f"kvs{p}")
            nc.vector.scalar_tensor_tensor(
                kvs[:], pkv[:, 0:128], 1.0, mdiag[:], OP.mult, OP.mult
            )
            # ks2[d, j] = ksum[d] * (head(d) == j)
            ks2 = ks2_p.tile([128, 2], BF16, tag="ks2", name=f"ks2{p}")
            nc.vector.scalar_tensor_tensor(
                ks2[:], pkv[:, 128:129].broadcast_to([128, 2]), 1.0,
                mask2T[:], OP.mult, OP.mult,
            )
            if DEBUG and b == 0:
                nc.sync.dma_start(io["kvs_dbg"][p][:, 0:128], kvs[:])
                nc.sync.dma_start(io["ks2_dbg"][p], ks2[:])
            st1.append((p, kvs, ks2, qfT, aT))
            if len(st1) > 4:
                numden_pair(st1.pop(0))
            if len(st2) > 3:
                finish_pair(st2.pop(0))

        if DEBUG and b == 0:
            nc.sync.dma_start(io["qfT_dbg"][:], qfT[:])

        # drain the attention tail interleaved with next batch's q phase
        if b + 1 < B_LOC:
            qfT_next = qfT_p.tile([128, NK, 512], BF16, tag="qfT",
                                  name=f"qfT{b + 1}")
            for oc in range(NK):
                q_oc(xT_next, qfT_next, oc)
                drain_step()
            drain_all()
            qfT = qfT_next
        else:
            drain_all()

        if b == 0:
            emit_bo2()

        if DEBUG and b == 0:
            nc.sync.dma_start(io["xT_dbg"][:], xT[:])
            nc.sync.dma_start(io["kf_dbg"][:], kf[:])
            nc.sync.dma_start(io["v_dbg"][:], vv[:])
            nc.sync.dma_start(io["aT_dbg"][:], aT[:])

        # ---- final projection ----
        for t in range(NT):
            pf = [ps.tile([128, 512], FP32, tag="ps", name=f"pf{h}") for h in range(2)]
            for ki in range(NK):
                lhs = aT[:, ki, t * 128:(t + 1) * 128]
                for h in range(2):
                    nc.tensor.matmul(
                        pf[h][:], lhsT=lhs,
                        rhs=W_bf["wo"][:, ki, h * 512:(h + 1) * 512],
                        start=(ki == 0), stop=False,
                    )
            osb = osb_p.tile([128, D], FP32, tag="osb")
            for h in range(2):
                nc.tensor.matmul(
                    pf[h][:], lhsT=ones_row[:, 0:128],
                    rhs=bo2_row[:, h * 512:(h + 1) * 512],
                    start=False, stop=True,
                )
                nc.any.tensor_copy(osb[:, h * 512:(h + 1) * 512], pf[h][:])
            nc.sync.dma_start(out_d[b, t * 128:(t + 1) * 128, :], osb[:])


_COMPILED = None
LAST_RESULTS = None
_PROFILE_READY = False


def _setup_profiling():
    """Best-effort: register the axon NTFF profile hook so that
    run_bass_kernel_spmd(trace=True) / BASS_TRACE=1 can report HW exec time.
    Silently no-ops if the environment doesn't support it."""
    global _PROFILE_READY
    if _PROFILE_READY:
        return
    _PROFILE_READY = True
    try:
        import sys
        import types

        try:
            from antenv.axon_hooks import get_axon_ntff_profile_hook  # noqa
            return  # already available
        except ImportError:
            pass

        import antenv

        mod = types.ModuleType("antenv.axon_hooks")
        mod._hook = None

        def set_axon_ntff_profile_hook(h):
            mod._hook = h

        def get_axon_ntff_profile_hook():
            return mod._hook

        mod.set_axon_ntff_profile_hook = set_axon_ntff_profile_hook
        mod.get_axon_ntff_profile_hook = get_axon_ntff_profile_hook
        sys.modules["antenv.axon_hooks"] = mod
        antenv.axon_hooks = mod

        if "/root/.axon_site" not in sys.path:
            sys.path.insert(0, "/root/.axon_site")
        from trn_agent_boot.trn_boot import _ntff_profile_via_ctypes

        so = "/opt/axon/libaxon_pjrt.so"
        if os.path.exists(so):
            hook = _ntff_profile_via_ctypes(so)
            if hook is not None:
                set_axon_ntff_profile_hook(hook)

        # artifact upload needs external storage; stub it out
        import concourse.bass_utils as bu

        bu.upload_artifacts = lambda tmpdir: f"local:{tmpdir}"
    except Exception:
        pass


def _build():
    nc = bacc.Bacc(
        "TRN2", target_bir_lowering=False, debug=False, num_devices=N_CORES
    )
    io = {
        "x": nc.dram_tensor("x", [B_LOC, N, D], FP32, kind="ExternalInput").ap(),
        "wq": nc.dram_tensor("wq", [D, D], FP32, kind="ExternalInput").ap(),
        "wk": nc.dram_tensor("wk", [D, D], FP32, kind="ExternalInput").ap(),
        "wv": nc.dram_tensor("wv", [D, D], FP32, kind="ExternalInput").ap(),
        "wo": nc.dram_tensor("wo", [D, D], FP32, kind="ExternalInput").ap(),
        "bq": nc.dram_tensor("bq", [1, D], FP32, kind="ExternalInput").ap(),
        "bk": nc.dram_tensor("bk", [1, D], FP32, kind="ExternalInput").ap(),
        "bv": nc.dram_tensor("bv", [NK, 128], FP32, kind="ExternalInput").ap(),
        "bo": nc.dram_tensor("bo", [1, D], FP32, kind="ExternalInput").ap(),
        "out": nc.dram_tensor("out", [B_LOC, N, D], FP32, kind="ExternalOutput").ap(),
    }
    if DEBUG:
        for nm, shp, dt in (
            ("xT_dbg", [128, NK, 512], BF16),
            ("qfT_dbg", [128, NK, 512], BF16),
            ("kf_dbg", [128, NT, D], BF16),
            ("v_dbg", [128, NT, 8 * 129], BF16),
            ("aT_dbg", [128, NK, 512], BF16),
            ("kvs_dbg", [NK, 128, 129], BF16),
            ("ks2_dbg", [NK, 128, 2], BF16),
            ("num_dbg", [NK, 128, 512], FP32),
            ("rc_dbg", [NK, 2, 512], FP32),
            ("bc_dbg", [NK, 128, 512], FP32),
        ):
            io[nm] = nc.dram_tensor(nm, shp, dt, kind="ExternalOutput").ap()
    with tile.TileContext(nc) as tc:
        _emit(tc, io)
    nc.compile()
    return nc


def get_nc():
    global _COMPILED
    if _COMPILED is None:
        _COMPILED = _build()
    return _COMPILED


def make_in_maps(x, W_q, b_q, W_k, b_k, W_v, b_v, W_o, b_o):
    x = np.asarray(x, dtype=np.float32)
    shared = {
        "wq": np.asarray(W_q, np.float32),
        "wk": np.asarray(W_k, np.float32),
        "wv": np.asarray(W_v, np.float32),
        "wo": np.asarray(W_o, np.float32),
        "bq": np.asarray(b_q, np.float32).reshape(1, D),
        "bk": np.asarray(b_k, np.float32).reshape(1, D),
        "bv": np.asarray(b_v, np.float32).reshape(NK, 128),
        "bo": np.asarray(b_o, np.float32).reshape(1, D),
    }
    return [
        {"x": np.ascontiguousarray(x[c * B_LOC:(c + 1) * B_LOC]), **shared}
        for c in range(N_CORES)
    ]


def kernel(x, W_q, b_q, W_k, b_k, W_v, b_v, W_o, b_o):
    global LAST_RESULTS
    if os.environ.get("BASS_TRACE"):
        _setup_profiling()
    nc = get_nc()
    in_maps = make_in_maps(x, W_q, b_q, W_k, b_k, W_v, b_v, W_o, b_o)
    res = run_bass_kernel_spmd(nc, in_maps, list(range(N_CORES)))
    LAST_RESULTS = res
    outs = [res.results[c]["out"] for c in range(N_CORES)]
    return np.concatenate(outs, axis=0).astype(np.float32)


# revision 67
# speedup vs baseline: 1.0218x; 1.0218x over previous
"""Linear-attention block (elu+1 feature map) for Trainium2, 8-core SPMD.

Data-parallel over batch: each of the 8 cores processes 4 of the 32 batches
with fully replicated weights; no collectives. All heavy matmuls run in bf16
with fp32 PSUM accumulation.

Layout strategy per core (B=4 local batches, N=512 tokens, D=1024, H=16,
Dk=64; heads processed as 8 "pairs" of 2 heads = 128 d_model dims):
  - x [128t,1024d] tiles -> cast bf16 -> DMA-xbar-transpose -> xT [d,t]
  - qT = W_q.T @ x.T   (W-chunk stationary)          [o,t] transposed
  - k,v = x @ W        (xT-chunk stationary)         [t,o] natural
  - biases via K=1 ones-row matmuls; b_v exactly folded into
    b_o2 = b_v @ W_o + b_o (the eps-correction term is ~1e-11, below fp32)
  - phi(y) = elu(y)+1 = min(exp(y),1) + relu(y): ACT exp + 2 DVE ops
  - per pair: kv[d,e]+ksum[d] one PSUM tile; numerator via two K=64
    tile_position matmuls on kv diagonal blocks; denominator via
    diag-masked ksum2 [128,2] matmul; 1/denom broadcast rows->128
    partitions with a constant 2x128 mask matmul; one DVE multiply
  - final = attn_outT.T @ W_o + b_o2 -> natural [t,o] -> DMA out
"""

import os
from contextlib import ExitStack

DEBUG = bool(int(os.environ.get("KERNEL_DEBUG", "0")))

import numpy as np

import concourse.bacc as bacc
import concourse.bass as bass
import concourse.mybir as mybir
import concourse.tile as tile
from concourse._compat import with_exitstack
from concourse.bass_utils import run_bass_kernel_spmd

FP32 = mybir.dt.float32
BF16 = mybir.dt.bfloat16
AF = mybir.ActivationFunctionType
OP = mybir.AluOpType

N_CORES = 8
B_LOC = 4          # batches per core
N = 512            # sequence length
D = 1024           # d_model
NT = 4             # token tiles of 128 per batch
NK = 8             # d_model chunks of 128
EPS = 1e-6


@with_exitstack
def _emit(ctx: ExitStack, tc: "tile.TileContext", io: dict):
    nc = tc.nc
    x_d = io["x"]
    out_d = io["out"]

    # ---------------- pools ----------------
    const = ctx.enter_context(tc.tile_pool(name="const", bufs=1))
    xbf_p = ctx.enter_context(tc.tile_pool(name="xbf", bufs=2))
    xT_p = ctx.enter_context(tc.tile_pool(name="xT", bufs=2))
    qfT_p = ctx.enter_context(tc.tile_pool(name="qfT", bufs=2))
    kf_p = ctx.enter_context(tc.tile_pool(name="kf", bufs=2))
    v_p = ctx.enter_context(tc.tile_pool(name="v", bufs=2))
    aT_p = ctx.enter_context(tc.tile_pool(name="aT", bufs=2))
    er_p = ctx.enter_context(tc.tile_pool(name="er", bufs=3))
    kvs_p = ctx.enter_context(tc.tile_pool(name="kvs", bufs=6))
    ks2_p = ctx.enter_context(tc.tile_pool(name="ks2", bufs=6))
    rc_p = ctx.enter_context(tc.tile_pool(name="rc", bufs=2))
    rcb_p = ctx.enter_context(tc.tile_pool(name="rcb", bufs=5))
    nsb_p = ctx.enter_context(tc.tile_pool(name="nsb", bufs=5))
    xstage_p = ctx.enter_context(tc.tile_pool(name="xstage", bufs=5))
    osb_p = ctx.enter_context(tc.tile_pool(name="osb", bufs=2))
    stage_p = ctx.enter_context(tc.tile_pool(name="wstage", bufs=2))
    ps = ctx.enter_context(tc.tile_pool(name="ps", bufs=6, space="PSUM"))
    pskv = ctx.enter_context(tc.tile_pool(name="pskv", bufs=2, space="PSUM"))

    # ---------------- constants ----------------
    ones_row = const.tile([1, 512], BF16, tag="ones_row")
    nc.vector.memset(ones_row[:], 1.0)
    one1 = const.tile([1, 1], BF16, tag="one1")
    nc.vector.memset(one1[:], 1.0)
    import ml_dtypes

    bf = ml_dtypes.bfloat16
    mask_np = np.zeros((2, 128), bf)
    mask_np[0, 0:64] = 1.0
    mask_np[1, 64:128] = 1.0
    mask2 = const.tile([2, 128], BF16, tag="mask2")
    nc.sync.dma_start(mask2[:], nc.inline_tensor(mask_np, name="mask2c").ap())
    # block-diagonal head mask [128, 128] and its [128, 2] column form
    mdiag_np = np.zeros((128, 128), bf)
    mdiag_np[0:64, 0:64] = 1.0
    mdiag_np[64:128, 64:128] = 1.0
    mdiag = const.tile([128, 128], BF16, tag="mdiag")
    nc.sync.dma_start(mdiag[:], nc.inline_tensor(mdiag_np, name="mdiagc").ap())
    m2t_np = np.zeros((128, 2), bf)
    m2t_np[0:64, 0] = 1.0
    m2t_np[64:128, 1] = 1.0
    mask2T = const.tile([128, 2], BF16, tag="mask2T")
    nc.sync.dma_start(mask2T[:], nc.inline_tensor(m2t_np, name="mask2Tc").ap())

    # ---------------- input loads ----------------
    # x: fp32 over HWDGE in halves + DVE cast (keeps the SWDGE path free for
    # W_q/W_k, which gate batch-0 compute)
    def load_x(b):
        xT = xT_p.tile([128, NK, 512], BF16, tag="xT", name=f"xT{b}")
        for t in range(NT):
            xb = xbf_p.tile([128, D], BF16, tag="xbf", name=f"xb{b}_{t}")
            for h in range(2):
                xs = xstage_p.tile([128, 512], FP32, tag="xstage",
                                   name=f"xs{b}_{t}_{h}")
                nc.sync.dma_start(
                    xs[:],
                    x_d[b, t * 128:(t + 1) * 128, h * 512:(h + 1) * 512],
                )
                nc.vector.tensor_copy(xb[:, h * 512:(h + 1) * 512], xs[:])
            # xbar transpose with partition-fold: out[p, c, t] = xb[t, c*128+p]
            # (on the ACT hwdge queue — the sync queue's serial descriptor
            # generation for big DMAs would delay these)
            nc.scalar.dma_start(
                xT[:, :, t * 128:(t + 1) * 128], xb[:], transpose=True
            )
        return xT

    def load_row(key):
        r = const.tile([1, D], BF16, tag=f"row_{key}", name=f"row_{key}")
        nc.gpsimd.dma_start(r[:], io[key][:])
        return r

    bk_row = load_row("bk")
    bo_row = load_row("bo")
    bv_cols = const.tile([128, NK], BF16, tag="bvc")
    nc.gpsimd.dma_start(bv_cols[:], io["bv"].rearrange("c p -> p c"))
    # b_q as column chunks (fp32) for ACT per-partition bias in the q phi
    bq_cols = const.tile([128, NK], FP32, tag="bqc")
    nc.gpsimd.dma_start(bq_cols[:], io["bq"].rearrange("r (c p) -> p (r c)", p=128))

    # Weights: W_q/W_k/W_o stream via gpsimd cast-DMAs; W_v goes fp32 over
    # HWDGE with DVE casts (parallel paths). Only W_q + x gate batch-0, so
    # the other loads are emitted later (= lower scheduler priority) to
    # leave them the HBM bandwidth.
    W_bf = {
        name: const.tile([128, NK, D], BF16, tag=f"W_{name}", name=f"W_{name}")
        for name in ("wq", "wk", "wv", "wo")
    }

    def emit_w_load(name):
        wd = io[name]
        wt = W_bf[name]
        for ki in range(NK):
            if name == "wv":
                ws = stage_p.tile([128, D], FP32, tag="wstage",
                                  name=f"ws_{name}{ki}")
                nc.sync.dma_start(ws[:], wd[ki * 128:(ki + 1) * 128, :])
                nc.vector.tensor_copy(wt[:, ki, :], ws[:])
            else:
                nc.gpsimd.dma_start(wt[:, ki, :], wd[ki * 128:(ki + 1) * 128, :])

    emit_w_load("wq")
    xT_next = load_x(0)

    bo2_row = const.tile([1, D], BF16, tag="bo2")

    def emit_bo2():
        # b_o2 = b_v @ W_o + b_o (emitted late so it doesn't head-of-line
        # block the in-order PE queue on the W_o DMA)
        for h in range(2):
            pb = ps.tile([128, 512], FP32, tag="ps", name=f"pbo2_{h}")
            for ki in range(NK):
                nc.tensor.matmul(
                    pb[0:1, :],
                    lhsT=bv_cols[:, ki:ki + 1],
                    rhs=W_bf["wo"][:, ki, h * 512:(h + 1) * 512],
                    start=(ki == 0),
                    stop=False,
                )
            nc.tensor.matmul(
                pb[0:1, :], lhsT=one1[:], rhs=bo_row[:, h * 512:(h + 1) * 512],
                start=False, stop=True,
            )
            nc.any.tensor_copy(bo2_row[:, h * 512:(h + 1) * 512], pb[0:1, :])

    # ---------------- phase emitters ----------------
    def q_oc(xT, qfT, oc):
        # one o-chunk of the transposed q projection + phi
        pq = ps.tile([128, 512], FP32, tag="ps", name=f"pq{oc}")
        for ki in range(NK):
            nc.tensor.matmul(
                pq[:],
                lhsT=W_bf["wq"][:, ki, oc * 128:(oc + 1) * 128],
                rhs=xT[:, ki, :],
                start=(ki == 0),
                stop=(ki == NK - 1),
            )
        e = er_p.tile([128, 512], FP32, tag="er", name=f"e{oc}")
        nc.scalar.activation(e[:], pq[:], AF.Exp, bias=bq_cols[:, oc:oc + 1])
        r = er_p.tile([128, 512], FP32, tag="er", name=f"r{oc}")
        nc.scalar.activation(r[:], pq[:], AF.Relu, bias=bq_cols[:, oc:oc + 1])
        nc.vector.scalar_tensor_tensor(
            qfT[:, oc, :], e[:], 1.0, r[:], OP.min, OP.add
        )

    # attention pipeline state (flows across batch boundaries)
    st1 = []  # (p, kvs, ks2, qfT, aT) awaiting num/den matmuls
    st2 = []  # (p, nsb, rcb, aT) awaiting bcast+div

    def numden_pair(pend):
        p, kvs, ks2, qfT, aT = pend
        pnum = ps.tile([128, 512], FP32, tag="ps", name=f"pnum{p}")
        nc.tensor.matmul(
            pnum[:], lhsT=kvs[:], rhs=qfT[:, p, :], start=True, stop=True
        )
        pden = ps.tile([2, 512], FP32, tag="ps", name=f"pden{p}")
        nc.tensor.matmul(
            pden[:], lhsT=ks2[:], rhs=qfT[:, p, :], start=True, stop=True
        )
        # eps (1e-6) dropped: denom = qf.ksum is >= O(100) here, so the
        # eps term is ~1e-9 relative — far below bf16 noise
        rc = rc_p.tile([2, 512], FP32, tag="rc", name=f"rc{p}")
        nc.vector.reciprocal_approx_fast(rc[:], pden[:])
        rcb = rcb_p.tile([2, 512], BF16, tag="rcb", name=f"rcb{p}")
        nc.vector.tensor_copy(rcb[:], rc[:])
        nsb = nsb_p.tile([128, 512], BF16, tag="nsb", name=f"nsb{p}")
        nc.scalar.copy(nsb[:], pnum[:])
        st2.append((p, nsb, rcb, aT))

    def finish_pair(pend):
        p, nsb, rcb, aT = pend
        pbc = ps.tile([128, 512], FP32, tag="ps", name=f"pbc{p}")
        nc.tensor.matmul(
            pbc[:], lhsT=mask2[:], rhs=rcb[:], start=True, stop=True
        )
        nc.vector.tensor_tensor(aT[:, p, :], nsb[:], pbc[:], OP.mult)

    def drain_step():
        if st1:
            numden_pair(st1.pop(0))
        if st2 and (len(st2) > 3 or not st1):
            finish_pair(st2.pop(0))

    def drain_all():
        while st1 or st2:
            drain_step()

    # ---------------- per-batch pipeline ----------------
    qfT = None
    for b in range(B_LOC):
        xT = xT_next
        if b + 1 < B_LOC:
            xT_next = load_x(b + 1)

        if b == 0:
            # ---- q (transposed) + phi (b_q folded into ACT bias) ----
            qfT = qfT_p.tile([128, NK, 512], BF16, tag="qfT", name="qfT0")
            for oc in range(NK):
                q_oc(xT, qfT, oc)
            # remaining weights, emitted after batch-0 q so their DMAs don't
            # steal HBM bandwidth from the startup-critical W_q + x loads
            emit_w_load("wk")
            emit_w_load("wv")
            emit_w_load("wo")

        # ---- k, v (natural) + phi / cast ----
        kf = kf_p.tile([128, NT, D], BF16, tag="kf")
        # v stored pair-strided: per pair p, cols [129p : 129p+128] = v,
        # col 129p+128 = 1.0 (so kv and ksum come from one matmul)
        vv = v_p.tile([128, NT, 8 * 129], BF16, tag="v")
        for t in range(NT):
            ones_cols = vv[:, t, :].rearrange("p (g c) -> p g c", c=129)[:, :, 128]
            nc.vector.memset(ones_cols, 1.0)
            pk = [ps.tile([128, 512], FP32, tag="ps", name=f"pk{h}") for h in range(2)]
            pv = [ps.tile([128, 512], FP32, tag="ps", name=f"pv{h}") for h in range(2)]
            for ki in range(NK):
                lhs = xT[:, ki, t * 128:(t + 1) * 128]
                for h in range(2):
                    nc.tensor.matmul(
                        pk[h][:], lhsT=lhs,
                        rhs=W_bf["wk"][:, ki, h * 512:(h + 1) * 512],
                        start=(ki == 0), stop=False,
                    )
                    nc.tensor.matmul(
                        pv[h][:], lhsT=lhs,
                        rhs=W_bf["wv"][:, ki, h * 512:(h + 1) * 512],
                        start=(ki == 0), stop=(ki == NK - 1),
                    )
            for h in range(2):
                nc.tensor.matmul(
                    pk[h][:], lhsT=ones_row[:, 0:128],
                    rhs=bk_row[:, h * 512:(h + 1) * 512],
                    start=False, stop=True,
                )
                e = er_p.tile([128, 512], FP32, tag="er")
                nc.scalar.activation(e[:], pk[h][:], AF.Exp)
                r = er_p.tile([128, 512], FP32, tag="er")
                nc.scalar.activation(r[:], pk[h][:], AF.Relu)
                nc.vector.scalar_tensor_tensor(
                    kf[:, t, h * 512:(h + 1) * 512], e[:], 1.0, r[:],
                    OP.min, OP.add,
                )
                for pl in range(4):
                    p = h * 4 + pl
                    nc.any.tensor_copy(
                        vv[:, t, p * 129:p * 129 + 128],
                        pv[h][:, pl * 128:(pl + 1) * 128],
                    )

        # ---- attention (8 head-pairs, two-level software pipeline: num/den
        # matmuls lag the kv matmuls by 3 pairs; bcast+div lag 2 more; the
        # tail drains interleaved with the NEXT batch's q projection so the
        # in-order PE queue never waits on ACT/DVE chains) ----
        aT = aT_p.tile([128, NK, 512], BF16, tag="aT")
        for p in range(NK):
            pkv = pskv.tile([128, 129], FP32, tag="pskv", name=f"pkv{p}")
            for t in range(NT):
                lhs = kf[:, t, p * 128:(p + 1) * 128]
                nc.tensor.matmul(
                    pkv[:], lhsT=lhs,
                    rhs=vv[:, t, p * 129:p * 129 + 129],
                    start=(t == 0), stop=(t == NT - 1),
                )
            # kvs = kv * blockdiag mask (kills cross-head cols), bf16
            kvs = kvs_p.tile([128, 128], BF16, tag="kvs", name=f"kvs{p}")
            nc.vector.scalar_tensor_tensor(
                kvs[:], pkv[:, 0:128], 1.0, mdiag[:], OP.mult, OP.mult
            )
            # ks2[d, j] = ksum[d] * (head(d) == j)
            ks2 = ks2_p.tile([128, 2], BF16, tag="ks2", name=f"ks2{p}")
            nc.vector.scalar_tensor_tensor(
                ks2[:], pkv[:, 128:129].broadcast_to([128, 2]), 1.0,
                mask2T[:], OP.mult, OP.mult,
            )
            if DEBUG and b == 0:
                nc.sync.dma_start(io["kvs_dbg"][p][:, 0:128], kvs[:])
                nc.sync.dma_start(io["ks2_dbg"][p], ks2[:])
            st1.append((p, kvs, ks2, qfT, aT))
            if len(st1) > 4:
                numden_pair(st1.pop(0))
            if len(st2) > 3:
                finish_pair(st2.pop(0))

        if DEBUG and b == 0:
            nc.sync.dma_start(io["qfT_dbg"][:], qfT[:])

        # drain the attention tail interleaved with next batch's q phase
        if b + 1 < B_LOC:
            qfT_next = qfT_p.tile([128, NK, 512], BF16, tag="qfT",
                                  name=f"qfT{b + 1}")
            for oc in range(NK):
                q_oc(xT_next, qfT_next, oc)
                drain_step()
            drain_all()
            qfT = qfT_next
        else:
            drain_all()

        if b == 0:
            emit_bo2()

        if DEBUG and b == 0:
            nc.sync.dma_start(io["xT_dbg"][:], xT[:])
            nc.sync.dma_start(io["kf_dbg"][:], kf[:])
            nc.sync.dma_start(io["v_dbg"][:], vv[:])
            nc.sync.dma_start(io["aT_dbg"][:], aT[:])

        # ---- final projection ----
        for t in range(NT):
            pf = [ps.tile([128, 512], FP32, tag="ps", name=f"pf{h}") for h in range(2)]
            for ki in range(NK):
                lhs = aT[:, ki, t * 128:(t + 1) * 128]
                for h in range(2):
                    nc.tensor.matmul(
                        pf[h][:], lhsT=lhs,
                        rhs=W_bf["wo"][:, ki, h * 512:(h + 1) * 512],
                        start=(ki == 0), stop=False,
                    )
            osb = osb_p.tile([128, D], FP32, tag="osb")
            for h in range(2):
                nc.tensor.matmul(
                    pf[h][:], lhsT=ones_row[:, 0:128],
                    rhs=bo2_row[:, h * 512:(h + 1) * 512],
                    start=False, stop=True,
                )
                nc.any.tensor_copy(osb[:, h * 512:(h + 1) * 512], pf[h][:])
            nc.sync.dma_start(out_d[b, t * 128:(t + 1) * 128, :], osb[:])


_COMPILED = None
LAST_RESULTS = None
_PROFILE_READY = False


def _setup_profiling():
    """Best-effort: register the axon NTFF profile hook so that
    run_bass_kernel_spmd(trace=True) / BASS_TRACE=1 can report HW exec time.
    Silently no-ops if the environment doesn't support it."""
    global _PROFILE_READY
    if _PROFILE_READY:
        return
    _PROFILE_READY = True
    try:
        import sys
        import types

        try:
            from antenv.axon_hooks import get_axon_ntff_profile_hook  # noqa
            return  # already available
        except ImportError:
            pass

        import antenv

        mod = types.ModuleType("antenv.axon_hooks")
        mod._hook = None

        def set_axon_ntff_profile_hook(h):
            mod._hook = h

        def get_axon_ntff_profile_hook():
            return mod._hook

        mod.set_axon_ntff_profile_hook = set_axon_ntff_profile_hook
        mod.get_axon_ntff_profile_hook = get_axon_ntff_profile_hook
        sys.modules["antenv.axon_hooks"] = mod
        antenv.axon_hooks = mod

        if "/root/.axon_site" not in sys.path:
            sys.path.insert(0, "/root/.axon_site")
        from trn_agent_boot.trn_boot import _ntff_profile_via_ctypes

        so = "/opt/axon/libaxon_pjrt.so"
        if os.path.exists(so):
            hook = _ntff_profile_via_ctypes(so)
            if hook is not None:
                set_axon_ntff_profile_hook(hook)

        # artifact upload needs external storage; stub it out
        import concourse.bass_utils as bu

        bu.upload_artifacts = lambda tmpdir: f"local:{tmpdir}"
    except Exception:
        pass


def _build():
    nc = bacc.Bacc(
        "TRN2", target_bir_lowering=False, debug=False, num_devices=N_CORES
    )
    io = {
        "x": nc.dram_tensor("x", [B_LOC, N, D], FP32, kind="ExternalInput").ap(),
        "wq": nc.dram_tensor("wq", [D, D], FP32, kind="ExternalInput").ap(),
        "wk": nc.dram_tensor("wk", [D, D], FP32, kind="ExternalInput").ap(),
        "wv": nc.dram_tensor("wv", [D, D], FP32, kind="ExternalInput").ap(),
        "wo": nc.dram_tensor("wo", [D, D], FP32, kind="ExternalInput").ap(),
        "bq": nc.dram_tensor("bq", [1, D], FP32, kind="ExternalInput").ap(),
        "bk": nc.dram_tensor("bk", [1, D], FP32, kind="ExternalInput").ap(),
        "bv": nc.dram_tensor("bv", [NK, 128], FP32, kind="ExternalInput").ap(),
        "bo": nc.dram_tensor("bo", [1, D], FP32, kind="ExternalInput").ap(),
        "out": nc.dram_tensor("out", [B_LOC, N, D], FP32, kind="ExternalOutput").ap(),
    }
    if DEBUG:
        for nm, shp, dt in (
            ("xT_dbg", [128, NK, 512], BF16),
            ("qfT_dbg", [128, NK, 512], BF16),
            ("kf_dbg", [128, NT, D], BF16),
            ("v_dbg", [128, NT, 8 * 129], BF16),
            ("aT_dbg", [128, NK, 512], BF16),
            ("kvs_dbg", [NK, 128, 129], BF16),
            ("ks2_dbg", [NK, 128, 2], BF16),
            ("num_dbg", [NK, 128, 512], FP32),
            ("rc_dbg", [NK, 2, 512], FP32),
            ("bc_dbg", [NK, 128, 512], FP32),
        ):
            io[nm] = nc.dram_tensor(nm, shp, dt, kind="ExternalOutput").ap()
    with tile.TileContext(nc) as tc:
        _emit(tc, io)
    nc.compile()
    return nc


def get_nc():
    global _COMPILED
    if _COMPILED is None:
        _COMPILED = _build()
    return _COMPILED


def make_in_maps(x, W_q, b_q, W_k, b_k, W_v, b_v, W_o, b_o):
    x = np.asarray(x, dtype=np.float32)
    shared = {
        "wq": np.asarray(W_q, np.float32),
        "wk": np.asarray(W_k, np.float32),
        "wv": np.asarray(W_v, np.float32),
        "wo": np.asarray(W_o, np.float32),
        "bq": np.asarray(b_q, np.float32).reshape(1, D),
        "bk": np.asarray(b_k, np.float32).reshape(1, D),
        "bv": np.asarray(b_v, np.float32).reshape(NK, 128),
        "bo": np.asarray(b_o, np.float32).reshape(1, D),
    }
    return [
        {"x": np.ascontiguousarray(x[c * B_LOC:(c + 1) * B_LOC]), **shared}
        for c in range(N_CORES)
    ]


def kernel(x, W_q, b_q, W_k, b_k, W_v, b_v, W_o, b_o):
    global LAST_RESULTS
    if os.environ.get("BASS_TRACE"):
        _setup_profiling()
    nc = get_nc()
    in_maps = make_in_maps(x, W_q, b_q, W_k, b_k, W_v, b_v, W_o, b_o)
    res = run_bass_kernel_spmd(nc, in_maps, list(range(N_CORES)))
    LAST_RESULTS = res
    outs = [res.results[c]["out"] for c in range(N_CORES)]
    return np.concatenate(outs, axis=0).astype(np.float32)
